# revision 6
# baseline (speedup 1.0000x reference)
"""Trainium2 Bass kernel for nn_LuminaLM (4-layer GPT-2-like transformer + LM head).

Strategy: 8-way Megatron tensor parallel with sequence-parallel residual.
 - Host precomputes embeddings (feature-major), folds LN gamma/beta into the
   consuming weights, casts all weights to bf16, and pre-transposes layouts so
   every DMA is contiguous per partition.
 - Each core owns 2 of 16 heads, 1/8 of the vocab; MLP is token-local
   (full fc weights streamed bf16) over the core's 256 tokens.
 - Residual h is token-sharded feature-major [128(dp), 8(dt), 128(t)] fp32.
 - Per layer-half: LN stats via ones-matmuls, normalize, AllGather bf16,
   qkv -> attention -> proj partial -> ReduceScatter bf16 -> residual add.
 - Attention: S^T computed directly per k-tile (k stationary, wide-N ragged),
   exp on ScalarE straight into SBUF P^T, causal zeroing via gpsimd
   affine_select, AV token-major with a fused ones-column giving row sums,
   softmax normalization via per-partition reciprocal, PE transpose to y^T.
 - LM head: activation-stationary (LDWEIGHTS amortized), vocab-sharded,
   token-major logits written fp32 via gpsimd-issued DMAs.
Matmuls are bf16 with fp32 PSUM accumulation; collectives ride bf16.
"""

import os
import numpy as np

B, T, D, V, L = 2, 1024, 1024, 32000, 4
H, HD = 16, 64
NCORES = 8
P = 128
TPC = T // NCORES          # 128 tokens per core per batch
HPC = H // NCORES          # 2 heads per core
QKVC = 3 * P               # 384 qkv cols per core (q:128, k:128, v:128)
VPC = V // NCORES          # 4000 vocab per core
MC = 125                   # lm-head vocab chunk (32 chunks of 125 = 4000)
NMC = VPC // MC            # 32
DT = D // P                # 8 d-tiles
NFC = 4 * D // P           # 32 fc1-output chunks
NT = T // 512              # 2 token chunks of 512 per half
EPS = 1e-5
ATT_SCALE = 1.0 / np.sqrt(HD)

_CACHE = {}
last_exec_time_ns = None


def _build_nc(no_coll=False):
    import concourse.bass as bass
    import concourse.mybir as mybir
    import concourse.tile as tile
    from concourse import bacc
    from concourse.masks import make_identity

    dt = mybir.dt
    AF = mybir.ActivationFunctionType
    OP = mybir.AluOpType

    nc = bacc.Bacc("TRN2", target_bir_lowering=False, debug=False,
                   num_devices=NCORES)

    # ---- external parameters (per-core shards, staged by host) ----
    emb_p = nc.declare_dram_parameter("emb_fm", [B, P, DT, TPC], dt.float32, isOutput=False)
    wqkv_p = nc.declare_dram_parameter("wqkv", [L, P, DT, QKVC], dt.bfloat16, isOutput=False)
    wproj_p = nc.declare_dram_parameter("wproj", [L, P, D], dt.bfloat16, isOutput=False)
    wfc1_p = nc.declare_dram_parameter("wfc1", [L, NFC, P, DT, P], dt.bfloat16, isOutput=False)
    wfc2_p = nc.declare_dram_parameter("wfc2", [L, DT, P, NFC, P], dt.bfloat16, isOutput=False)
    wlm_p = nc.declare_dram_parameter("wlm", [NMC, P, DT, MC], dt.bfloat16, isOutput=False)
    bias_p = nc.declare_dram_parameter("bias_all", [L, P, 3 + NFC + DT + DT], dt.float32, isOutput=False)
    blm_p = nc.declare_dram_parameter("blm", [MC, NMC], dt.float32, isOutput=False)
    logits_p = nc.declare_dram_parameter("logits", [VPC, B * T], dt.float32, isOutput=True)

    RG = [list(range(NCORES))]

    with tile.TileContext(nc) as tc:
        with (
            tc.tile_pool(name="const", bufs=1) as cp,
            tc.tile_pool(name="wp", bufs=2) as wp,
            tc.tile_pool(name="ap", bufs=2) as app,
            tc.tile_pool(name="psA", bufs=3, space="PSUM") as psA,
            tc.tile_pool(name="psS", bufs=2, space="PSUM") as psS,
            tc.tile_pool(name="psY", bufs=3, space="PSUM") as psY,
            tc.tile_pool(name="dram", bufs=2, space="DRAM") as dramp,
        ):
            # ---------------- constants ----------------
            ident_bf = cp.tile([P, P], dt.bfloat16)
            make_identity(nc, ident_bf[:])
            ones_col_bf = cp.tile([P, 1], dt.bfloat16)
            nc.any.memset(ones_col_bf[:], 1.0)
            ones_row_f = cp.tile([1, P], dt.float32)
            nc.any.memset(ones_row_f[:], 1.0)
            ones_row_bf = cp.tile([1, P], dt.bfloat16)
            nc.any.memset(ones_row_bf[:], 1.0)
            eps_sb = cp.tile([1, 1], dt.float32)
            nc.any.memset(eps_sb[:], EPS)

            # all per-layer biases in one tile [P, L, 61]
            NB = 3 + NFC + DT + DT
            bias_sb = cp.tile([P, L, NB], dt.float32)
            nc.sync.dma_start(bias_sb[:], bias_p[:].rearrange("l p c -> p l c"))

            def bqkvt(li):
                return bias_sb[:, li, 0:3]

            def bfc1t(li):
                return bias_sb[:, li, 3:3 + NFC]

            def bprojt(li):
                return bias_sb[:, li, 3 + NFC:3 + NFC + DT]

            def bfc2t(li):
                return bias_sb[:, li, 3 + NFC + DT:NB]

            blm_sb = cp.tile([MC, NMC], dt.float32)
            nc.sync.dma_start(blm_sb[:], blm_p[:])

            # tiny warm-up AllGather: absorbs cross-core kernel-start skew
            # while the embedding DMAs and first LN run
            if not no_coll:
                wu_sb = cp.tile([1, 16], dt.float32)
                nc.any.memset(wu_sb[:], 0.0)
                wu_in = dramp.tile([1, 16], dt.float32, name="wu_in", tag="wu_in",
                                   bufs=1)
                wu_out = dramp.tile([NCORES, 16], dt.float32, name="wu_out",
                                    tag="wu_out", bufs=1, addr_space="Shared")
                nc.sync.dma_start(wu_in[:], wu_sb[:])
                nc.gpsimd.collective_compute(
                    "AllGather", OP.bypass, replica_groups=RG,
                    ins=[wu_in[:].opt()], outs=[wu_out[:].opt()],
                )

            # ---------------- embedding ----------------
            hres = [cp.tile([P, DT, TPC], dt.float32, name=f"hres{h}") for h in range(B)]
            for half in range(B):
                nc.sync.dma_start(hres[half][:], emb_p[half])

            # ---------------- layernorm ----------------
            def ln_normalize(h_tile, name, out_ap=None):
                """Token-shard LN without gamma/beta (folded into weights).
                Returns bf16 [P, DT, TPC] tile of (h-m)*rstd (or writes into
                out_ap [P, DT, TPC] if given)."""
                hb = app.tile([P, DT, TPC], dt.bfloat16, name=f"hb_{name}", tag="hb")
                nc.vector.tensor_copy(hb[:], h_tile[:])
                hb2 = app.tile([P, DT, TPC], dt.bfloat16, name=f"hb2_{name}",
                               tag="hb2", bufs=2)
                nc.vector.tensor_mul(hb2[:], hb[:], hb[:])
                ps_sum = psY.tile([1, TPC], dt.float32, space="PSUM",
                                  name=f"psum_{name}", tag="small")
                ps_sq = psY.tile([1, TPC], dt.float32, space="PSUM",
                                 name=f"psq_{name}", tag="small")
                for dti in range(DT):
                    nc.tensor.matmul(ps_sum[:], lhsT=ones_col_bf[:], rhs=hb[:, dti, :],
                                     start=(dti == 0), stop=(dti == DT - 1))
                for dti in range(DT):
                    nc.tensor.matmul(ps_sq[:], lhsT=ones_col_bf[:], rhs=hb2[:, dti, :],
                                     start=(dti == 0), stop=(dti == DT - 1))
                m_sb = app.tile([1, TPC], dt.float32, name=f"m_{name}", tag="m")
                nc.vector.tensor_scalar_mul(m_sb[:], ps_sum[:], 1.0 / D)
                mm_sb = app.tile([1, TPC], dt.float32, name=f"mm_{name}", tag="mm")
                nc.vector.tensor_mul(mm_sb[:], m_sb[:], m_sb[:])
                var_sb = app.tile([1, TPC], dt.float32, name=f"var_{name}", tag="var")
                nc.vector.scalar_tensor_tensor(
                    out=var_sb[:], in0=ps_sq[:], scalar=1.0 / D, in1=mm_sb[:],
                    op0=OP.mult, op1=OP.subtract)
                std_sb = app.tile([1, TPC], dt.float32, name=f"std_{name}", tag="std")
                nc.scalar.activation(std_sb[:], var_sb[:], AF.Sqrt, bias=eps_sb[:])
                rstd_sb = app.tile([1, TPC], dt.float32, name=f"rstd_{name}", tag="rstd")
                nc.vector.reciprocal(rstd_sb[:], std_sb[:])
                mrstd_sb = app.tile([1, TPC], dt.float32, name=f"mrstd_{name}", tag="mrstd")
                nc.vector.scalar_tensor_tensor(
                    out=mrstd_sb[:], in0=m_sb[:], scalar=-1.0, in1=rstd_sb[:],
                    op0=OP.mult, op1=OP.mult)
                # broadcast across partitions via K=1 fp32 matmuls
                ps_r = psY.tile([P, TPC], dt.float32, space="PSUM",
                                name=f"psr_{name}", tag="small")
                nc.tensor.matmul(ps_r[:], lhsT=ones_row_f[:], rhs=rstd_sb[:],
                                 start=True, stop=True)
                rstd_full = app.tile([P, TPC], dt.bfloat16, name=f"rstdf_{name}", tag="rstdf")
                nc.vector.tensor_copy(rstd_full[:], ps_r[:])
                ps_mr = psY.tile([P, TPC], dt.float32, space="PSUM",
                                 name=f"psmr_{name}", tag="small")
                nc.tensor.matmul(ps_mr[:], lhsT=ones_row_f[:], rhs=mrstd_sb[:],
                                 start=True, stop=True)
                mrstd_full = app.tile([P, TPC], dt.bfloat16, name=f"mrstdf_{name}", tag="mrstdf")
                nc.vector.tensor_copy(mrstd_full[:], ps_mr[:])
                t1 = app.tile([P, DT, TPC], dt.bfloat16, name=f"t1_{name}",
                              tag="t1", bufs=2)
                nc.vector.tensor_tensor(
                    out=t1[:], in0=hb[:],
                    in1=rstd_full[:, None, :].to_broadcast([P, DT, TPC]), op=OP.mult)
                if out_ap is None:
                    hn = app.tile([P, DT, TPC], dt.bfloat16, name=f"hn_{name}",
                                  tag="hn", bufs=3)
                    out_ap = hn[:]
                else:
                    hn = None
                nc.vector.tensor_tensor(
                    out=out_ap, in0=t1[:],
                    in1=mrstd_full[:, None, :].to_broadcast([P, DT, TPC]), op=OP.add)
                return hn

            # ---------------- collectives ----------------
            def allgather_read(hn, name):
                """AllGather one half's LN'd shard; returns two aT tiles
                [P, 4, DT, TPC] bf16 (global token blocks 0-511 / 512-1023)
                so qkv can start as soon as the first block lands."""
                ag_in = dramp.tile([P * DT, TPC], dt.bfloat16,
                                   name=f"agin_{name}", tag="agin")
                nc.sync.dma_start(
                    ag_in[:].rearrange("(p dt) t -> p dt t", p=P), hn[:])
                ag_out = dramp.tile([NCORES * P * DT, TPC], dt.bfloat16,
                                    name=f"agout_{name}", tag="agout",
                                    addr_space="Shared")
                if no_coll:
                    nc.sync.dma_start(ag_out[0:P * DT, :], ag_in[:])
                else:
                    nc.gpsimd.collective_compute(
                        "AllGather", OP.bypass, replica_groups=RG,
                        ins=[ag_in[:].opt()], outs=[ag_out[:].opt()],
                    )
                ag_view = ag_out[:].rearrange(
                    "(rb r p dt) t -> rb p r (dt t)", rb=2, p=P, dt=DT)
                aTs = []
                for rb in range(2):
                    aT = app.tile([P, 4, DT * TPC], dt.bfloat16,
                                  name=f"aT_{name}{rb}", tag="aT", bufs=4)
                    nc.sync.dma_start(aT[:], ag_view[rb])
                    aTs.append(aT.rearrange("p r (dt t) -> p r dt t", dt=DT))
                return aTs

            def reduce_scatter_residual(rs_in, li, h_tile, name):
                """rs_in: DRAM [NCORES, P, DT, TPC] bf16 feature-major partials
                blocked by destination core. RS, add partial+bias into residual."""
                rs_out = dramp.tile([P * DT, TPC], dt.bfloat16, name=f"rsout_{name}",
                                    tag="rsout")
                if no_coll:
                    nc.sync.dma_start(
                        rs_out[:], rs_in[0].rearrange("p dt t -> (p dt) t"))
                else:
                    nc.gpsimd.collective_compute(
                        "ReduceScatter", OP.add, replica_groups=RG,
                        ins=[rs_in[:].rearrange("r p dt t -> (r p dt) t").opt()],
                        outs=[rs_out[:].opt()],
                    )
                rsb = app.tile([P, DT, TPC], dt.bfloat16, name=f"rsb_{name}", tag="rsb")
                nc.sync.dma_start(rsb[:], rs_out[:].rearrange("(p dc) t -> p dc t", p=P))
                bias_t = bprojt(li)
                for dc in range(DT):
                    nc.vector.scalar_tensor_tensor(
                        out=h_tile[:, dc, :], in0=rsb[:, dc, :],
                        scalar=bias_t[:, dc:dc + 1], in1=h_tile[:, dc, :],
                        op0=OP.add, op1=OP.add)

            # ---------------- layer blocks ----------------
            def load_weights(li):
                wqkv = wp.tile([P, DT, QKVC], dt.bfloat16, name=f"wqkv{li}", tag="wqkv")
                nc.sync.dma_start(wqkv[:], wqkv_p[li])
                wproj = wp.tile([P, D], dt.bfloat16, name=f"wproj{li}", tag="wproj")
                nc.sync.dma_start(wproj[:], wproj_p[li])
                return wqkv, wproj

            def qkv_block(aT2, wqkv, li, half):
                """q,k,v feature-major [P, 3, T] bf16 (+bias)."""
                qkT = app.tile([P, 3, T], dt.bfloat16, name=f"qkT{half}", tag="qkT")
                bq = bqkvt(li)
                for tk in range(NT):
                    for c in range(3):
                        ps = psA.tile([P, 512], dt.float32, space="PSUM",
                                      name="ps_qkv", tag="psA")
                        for dti in range(DT):
                            nc.tensor.matmul(
                                ps[:], lhsT=wqkv[:, dti, c * P:(c + 1) * P],
                                rhs=aT2[tk][:, :, dti, :],
                                start=(dti == 0), stop=(dti == DT - 1))
                        if c % 2 == 0:
                            nc.vector.tensor_scalar_add(
                                qkT[:, c, tk * 512:(tk + 1) * 512], ps[:],
                                bq[:, c:c + 1])
                        else:
                            nc.scalar.add(
                                qkT[:, c, tk * 512:(tk + 1) * 512], ps[:],
                                bq[:, c:c + 1])
                return qkT

            def attention(qkT, half):
                # v -> token-major [128(t), 8(tt), 2(h2), 65] with a ones col
                v_tok = app.tile([P, DT, HPC, HD + 1], dt.bfloat16,
                                 name=f"vtok{half}", tag="vtok")
                nc.any.memset(v_tok[:, :, :, HD:HD + 1], 1.0)
                for tt in range(DT):
                    pst = psY.tile([P, P], dt.bfloat16, space="PSUM", name="pst_v",
                                   tag="small")
                    nc.tensor.transpose(
                        pst[:], qkT[:, 2, tt * P:(tt + 1) * P], ident_bf[:])
                    nc.vector.tensor_copy(
                        v_tok[:, tt, :, 0:HD],
                        pst[:].rearrange("p (h2 d) -> p h2 d", h2=HPC))

                y_sb = app.tile([P, DT, P], dt.bfloat16, name=f"ysb{half}", tag="ysb")
                yT = app.tile([P, T], dt.bfloat16, name=f"yT{half}", tag="yT")
                for qc in range(NT):  # 512-query chunks
                    PTs = [app.tile([P, 8, 512], dt.bfloat16,
                                    name=f"PT{half}_{qc}_{h2}", tag=f"PT{h2}",
                                    bufs=1)
                           for h2 in range(HPC)]
                    for h2 in range(HPC):
                        hs = h2 * HD
                        nkt = qc * 4 + 4
                        for kt in range(nkt):
                            off = max(0, kt * P - qc * 512)
                            ps_st = psS.tile([P, 512], dt.float32, space="PSUM",
                                             name="ps_st", tag="psS")
                            nc.tensor.matmul(
                                ps_st[:, off:512],
                                lhsT=qkT[hs:hs + HD, 1, kt * P:(kt + 1) * P],
                                rhs=qkT[hs:hs + HD, 0,
                                        qc * 512 + off:(qc + 1) * 512],
                                start=True, stop=True)
                            nc.scalar.activation(
                                PTs[h2][:, kt, off:512], ps_st[:, off:512],
                                AF.Exp, scale=ATT_SCALE)
                            if kt >= qc * 4:
                                # causal zeroing of the diagonal 128-sub-tile:
                                # keep exp where q' - k' >= 0, else 0
                                nc.gpsimd.affine_select(
                                    out=PTs[h2][:, kt, off:off + P],
                                    in_=PTs[h2][:, kt, off:off + P],
                                    compare_op=OP.is_ge, fill=0.0, base=0,
                                    pattern=[[1, P]], channel_multiplier=-1)
                    # AV token-major, fused row-sum via the ones column
                    for qt in range(qc * 4, qc * 4 + 4):
                        qoff = qt * P - qc * 512
                        for h2 in range(HPC):
                            hs = h2 * HD
                            ps_y = psY.tile([P, HD + 1], dt.float32, space="PSUM",
                                            name="ps_y", tag="small")
                            for kt in range(qt + 1):
                                nc.tensor.matmul(
                                    ps_y[:], lhsT=PTs[h2][:, kt, qoff:qoff + P],
                                    rhs=v_tok[:, kt, h2, :],
                                    start=(kt == 0), stop=(kt == qt))
                            rec = app.tile([P, 1], dt.float32, name="rec", tag="rec",
                                           bufs=3)
                            nc.vector.reciprocal(rec[:], ps_y[:, HD:HD + 1])
                            nc.vector.tensor_scalar_mul(
                                y_sb[:, qt, hs:hs + HD], ps_y[:, 0:HD], rec[:, 0:1])
                    for qt in range(qc * 4, qc * 4 + 4):
                        ps_t = psY.tile([P, P], dt.bfloat16, space="PSUM",
                                        name="ps_t", tag="small")
                        nc.tensor.transpose(ps_t[:], y_sb[:, qt, :], ident_bf[:])
                        nc.vector.tensor_copy(yT[:, qt * P:(qt + 1) * P], ps_t[:])
                return yT

            def proj_partial(yT, wproj, half, name):
                """Partial attn output, feature-major, blocked by destination
                core: rs_in [NCORES, P, DT, TPC] bf16."""
                rs_in = dramp.tile([NCORES, P, DT, TPC], dt.bfloat16,
                                   name=f"rsin_{name}", tag="rsin")
                for tk in range(NT):
                    prd = app.tile([P, DT, 512], dt.bfloat16, name=f"prd{half}_{tk}",
                                   tag="prd")
                    for dc in range(DT):
                        ps = psA.tile([P, 512], dt.float32, space="PSUM",
                                      name="ps_pr", tag="psA")
                        nc.tensor.matmul(
                            ps[:], lhsT=wproj[:, dc * P:(dc + 1) * P],
                            rhs=yT[:, tk * 512:(tk + 1) * 512], start=True, stop=True)
                        if dc % 2 == 0:
                            nc.vector.tensor_copy(prd[:, dc, :], ps[:])
                        else:
                            nc.scalar.copy(prd[:, dc, :], ps[:])
                    for tb in range(tk * 4, (tk + 1) * 4):
                        nc.sync.dma_start(
                            rs_in[tb], prd[:, :, (tb - tk * 4) * P:(tb - tk * 4 + 1) * P])
                return rs_in

            def mlp_half(hn2, li, half):
                """Token-local MLP for one half (128 tokens) with streamed
                bf16 fc weights. Run per half so one half's MLP hides the
                other half's collectives."""
                mTm = app.tile([P, NFC, TPC], dt.bfloat16, name=f"mTm{li}_{half}",
                               tag="mTm")
                b1 = bfc1t(li)
                for fc in range(NFC):
                    wf1c = wp.tile([P, DT, P], dt.bfloat16,
                                   name=f"wf1c{li}_{half}_{fc}", tag="wf1c",
                                   bufs=4)
                    nc.sync.dma_start(wf1c[:], wfc1_p[li, fc])
                    ps = psA.tile([P, TPC], dt.float32, space="PSUM",
                                  name="ps_f1", tag="psA")
                    for dti in range(DT):
                        nc.tensor.matmul(
                            ps[:], lhsT=wf1c[:, dti, :], rhs=hn2[:, dti, :],
                            start=(dti == 0), stop=(dti == DT - 1))
                    nc.scalar.activation(
                        mTm[:, fc, :], ps[:], AF.Gelu, bias=b1[:, fc:fc + 1])
                b2 = bfc2t(li)
                for dc in range(DT):
                    wf2c = wp.tile([P, NFC, P], dt.bfloat16,
                                   name=f"wf2c{li}_{half}_{dc}", tag="wf2c",
                                   bufs=2)
                    nc.sync.dma_start(wf2c[:], wfc2_p[li, dc])
                    ps2 = psA.tile([P, TPC], dt.float32, space="PSUM",
                                   name="ps_f2", tag="psA")
                    for kt in range(NFC):
                        nc.tensor.matmul(
                            ps2[:], lhsT=wf2c[:, kt, :], rhs=mTm[:, kt, :],
                            start=(kt == 0), stop=(kt == NFC - 1))
                    nc.vector.scalar_tensor_tensor(
                        out=hres[half][:, dc, :], in0=ps2[:],
                        scalar=b2[:, dc:dc + 1],
                        in1=hres[half][:, dc, :], op0=OP.add, op1=OP.add)

            # ---------------- transformer layers ----------------
            # the two batch halves run as independent software-pipelined
            # streams: while one half waits on its AllGather/ReduceScatter,
            # the other half's attention/MLP keeps the PE busy
            ag = []
            for h in range(B):
                hn0 = ln_normalize(hres[h], f"pro{h}")
                ag.append(allgather_read(hn0, f"l0a{h}"))
            for li in range(L):
                wqkv, wproj = load_weights(li)
                ag_next = [None, None]
                for h in range(B):
                    qkT = qkv_block(ag[h], wqkv, li, h)
                    yT = attention(qkT, h)
                    rs_in = proj_partial(yT, wproj, h, f"l{li}p{h}")
                    reduce_scatter_residual(rs_in, li, hres[h], f"l{li}p{h}")
                    hn2 = ln_normalize(hres[h], f"l{li}m{h}")
                    mlp_half(hn2, li, h)
                    # trailing LN + AllGather feeds the next layer (or, after
                    # the last layer, the folded final LN for the LM head)
                    hn1 = ln_normalize(hres[h], f"l{li}n{h}")
                    ag_next[h] = allgather_read(hn1, f"l{li + 1}a{h}")
                ag = ag_next

            # ---------------- LM head ----------------
            afTs = ag
            for mc in range(NMC):
                wlm = wp.tile([P, DT, MC], dt.bfloat16, name=f"wlm{mc}", tag="wlm",
                              bufs=3)
                nc.sync.dma_start(wlm[:], wlm_p[mc])
                # per pass: two 512-token chunks accumulate in two PSUM banks;
                # consecutive matmuls share the same stationary wlm slice so
                # the LDWEIGHTS pipeline stays fed
                for hp in range(B):
                    psL = [psA.tile([MC, 512], dt.float32, space="PSUM",
                                    name=f"ps_lm{mc}_{hp}_{i}", tag="psA")
                           for i in range(2)]
                    for dti in range(DT):
                        for tq in range(2):
                            nc.tensor.matmul(
                                psL[tq][:],
                                lhsT=wlm[:, dti, :],
                                rhs=afTs[hp][tq][:, :, dti, :],
                                start=(dti == 0), stop=(dti == DT - 1))
                    for tq in range(2):
                        lsb = app.tile([MC, 512], dt.float32, name="lsb",
                                       tag="lsb", bufs=4)
                        if tq == 0:
                            nc.vector.tensor_scalar_add(
                                lsb[:], psL[tq][:], blm_sb[:, mc:mc + 1])
                        else:
                            nc.scalar.add(lsb[:], psL[tq][:], blm_sb[:, mc:mc + 1])
                        nc.gpsimd.dma_start(
                            logits_p[mc * MC:(mc + 1) * MC,
                                     hp * T + tq * 512:hp * T + (tq + 1) * 512],
                            lsb[:])

    nc.compile()
    return nc


def _get_nc():
    no_coll = os.environ.get("KERNEL_NO_COLL", "0") == "1"
    key = ("nc", no_coll)
    if key not in _CACHE:
        _CACHE[key] = _build_nc(no_coll)
    return _CACHE[key]


def build_in_maps(input_ids, wte, wpe, ln1_g, ln1_b, w_qkv, b_qkv, w_proj,
                  b_proj, ln2_g, ln2_b, w_fc1, b_fc1, w_fc2, b_fc2, lnf_g,
                  lnf_b, w_lm):
    import ml_dtypes
    f32 = np.float32
    bf16 = ml_dtypes.bfloat16

    ids = np.asarray(input_ids).astype(np.int64)
    wte = np.asarray(wte, dtype=f32)
    wpe = np.asarray(wpe, dtype=f32)
    g1 = np.asarray(ln1_g, f32)
    b1 = np.asarray(ln1_b, f32)
    g2 = np.asarray(ln2_g, f32)
    b2 = np.asarray(ln2_b, f32)
    gf = np.asarray(lnf_g, f32)
    bf = np.asarray(lnf_b, f32)
    Wq = np.asarray(w_qkv, f32)
    Wp = np.asarray(w_proj, f32)
    W1 = np.asarray(w_fc1, f32)
    W2 = np.asarray(w_fc2, f32)
    Wlm = np.asarray(w_lm, f32)
    bq = np.asarray(b_qkv, f32)
    bp = np.asarray(b_proj, f32)
    bb1 = np.asarray(b_fc1, f32)
    bb2 = np.asarray(b_fc2, f32)

    # fold LN gains into consuming weights; betas into their biases
    Wq_f = Wq * g1[:, :, None]                       # [L, D, 3D]
    bq_f = np.einsum('ld,ldo->lo', b1, Wq) + bq      # [L, 3D]
    W1_f = W1 * g2[:, :, None]                       # [L, D, 4D]
    b1_f = np.einsum('ld,ldo->lo', b2, W1) + bb1     # [L, 4D]
    Wlm_f = Wlm * gf[:, None]                        # [D, V]
    blm_f = bf @ Wlm                                 # [V]

    # embeddings, feature-major per core
    emb = wte[ids] + wpe[None, :, :]                 # [B, T, D]

    in_maps = []
    for r in range(NCORES):
        t0, t1 = r * TPC, (r + 1) * TPC
        cols = np.r_[P * r:P * r + P, D + P * r:D + P * r + P,
                     2 * D + P * r:2 * D + P * r + P]
        vs, ve = r * VPC, (r + 1) * VPC

        # emb_fm [B, P, DT, TPC]
        e = emb[:, t0:t1, :]                         # [B, TPC, D]
        emb_fm = np.ascontiguousarray(
            e.transpose(0, 2, 1).reshape(B, DT, P, TPC).transpose(0, 2, 1, 3))

        # wqkv [L, P, DT, QKVC]
        wq = Wq_f[:, :, cols]                        # [L, D, 384]
        wq = wq.reshape(L, DT, P, QKVC).transpose(0, 2, 1, 3)

        # wproj [L, P, D] (rows P*r..P*r+P)
        wpj = Wp[:, P * r:P * r + P, :]

        # wfc1 [L, NFC, P, DT, P]
        w1 = W1_f.reshape(L, DT, P, NFC, P).transpose(0, 3, 2, 1, 4)

        # wfc2 [L, DT, P, NFC, P]
        w2 = W2.reshape(L, NFC, P, DT, P).transpose(0, 3, 2, 1, 4)

        # wlm [NMC, P, DT, MC]
        wl = Wlm_f[:, vs:ve].reshape(DT, P, NMC, MC).transpose(2, 1, 0, 3)

        # bias_all [L, P, 3 + NFC + DT + DT]
        bias_all = np.concatenate([
            bq_f[:, cols].reshape(L, 3, P).transpose(0, 2, 1),
            b1_f.reshape(L, NFC, P).transpose(0, 2, 1),
            bp.reshape(L, DT, P).transpose(0, 2, 1),
            bb2.reshape(L, DT, P).transpose(0, 2, 1),
        ], axis=2)

        m = {
            "emb_fm": emb_fm,
            "wqkv": np.ascontiguousarray(wq.astype(bf16)),
            "wproj": np.ascontiguousarray(wpj.astype(bf16)),
            "wfc1": np.ascontiguousarray(w1.astype(bf16)),
            "wfc2": np.ascontiguousarray(w2.astype(bf16)),
            "wlm": np.ascontiguousarray(wl.astype(bf16)),
            "bias_all": np.ascontiguousarray(bias_all),
            "blm": np.ascontiguousarray(
                blm_f[vs:ve].reshape(NMC, MC).T.astype(f32)),
        }
        in_maps.append(m)

    return in_maps


def kernel(**inputs):
    global last_exec_time_ns
    from concourse.bass_utils import run_bass_kernel_spmd

    in_maps = build_in_maps(**inputs)
    nc = _get_nc()
    trace = os.environ.get("KERNEL_TRACE", "0") == "1"
    res = run_bass_kernel_spmd(nc, in_maps, list(range(NCORES)), trace=trace)
    last_exec_time_ns = res.exec_time_ns

    parts = [res.results[r]["logits"] for r in range(NCORES)]  # [VPC, B*T] each
    full = np.concatenate(parts, axis=0)          # [V, B*T]
    out = full.T.reshape(B, T, V).astype(np.float32)
    return out


# revision 7
# speedup vs baseline: 1.0091x; 1.0091x over previous
"""Trainium2 Bass kernel for nn_LuminaLM (4-layer GPT-2-like transformer + LM head).

Strategy: 8-way Megatron tensor parallel with sequence-parallel residual.
 - Host precomputes embeddings (feature-major), folds LN gamma/beta into the
   consuming weights, casts all weights to bf16, and pre-transposes layouts so
   every DMA is contiguous per partition.
 - Each core owns 2 of 16 heads, 1/8 of the vocab; MLP is token-local
   (full fc weights streamed bf16) over the core's 256 tokens.
 - Residual h is token-sharded feature-major [128(dp), 8(dt), 128(t)] fp32.
 - Per layer-half: LN stats via ones-matmuls, normalize, AllGather bf16,
   qkv -> attention -> proj partial -> ReduceScatter bf16 -> residual add.
 - Attention: S^T computed directly per k-tile (k stationary, wide-N ragged),
   exp on ScalarE straight into SBUF P^T, causal zeroing via gpsimd
   affine_select, AV token-major with a fused ones-column giving row sums,
   softmax normalization via per-partition reciprocal, PE transpose to y^T.
 - LM head: activation-stationary (LDWEIGHTS amortized), vocab-sharded,
   token-major logits written fp32 via gpsimd-issued DMAs.
Matmuls are bf16 with fp32 PSUM accumulation; collectives ride bf16.
"""

import os
import numpy as np

B, T, D, V, L = 2, 1024, 1024, 32000, 4
H, HD = 16, 64
NCORES = 8
P = 128
TPC = T // NCORES          # 128 tokens per core per batch
HPC = H // NCORES          # 2 heads per core
QKVC = 3 * P               # 384 qkv cols per core (q:128, k:128, v:128)
VPC = V // NCORES          # 4000 vocab per core
MC = 125                   # lm-head vocab chunk (32 chunks of 125 = 4000)
NMC = VPC // MC            # 32
DT = D // P                # 8 d-tiles
NFC = 4 * D // P           # 32 fc1-output chunks
NT = T // 512              # 2 token chunks of 512 per half
EPS = 1e-5
ATT_SCALE = 1.0 / np.sqrt(HD)

_CACHE = {}
last_exec_time_ns = None


def _build_nc(no_coll=False):
    import concourse.bass as bass
    import concourse.mybir as mybir
    import concourse.tile as tile
    from concourse import bacc
    from concourse.masks import make_identity

    dt = mybir.dt
    AF = mybir.ActivationFunctionType
    OP = mybir.AluOpType

    nc = bacc.Bacc("TRN2", target_bir_lowering=False, debug=False,
                   num_devices=NCORES)

    # ---- external parameters (per-core shards, staged by host) ----
    emb_p = nc.declare_dram_parameter("emb_fm", [B, P, DT, TPC], dt.float32, isOutput=False)
    wqkv_p = nc.declare_dram_parameter("wqkv", [L, P, DT, QKVC], dt.bfloat16, isOutput=False)
    wproj_p = nc.declare_dram_parameter("wproj", [L, P, D], dt.bfloat16, isOutput=False)
    wfc1_p = nc.declare_dram_parameter("wfc1", [L, NFC, P, DT, P], dt.bfloat16, isOutput=False)
    wfc2_p = nc.declare_dram_parameter("wfc2", [L, DT, P, NFC, P], dt.bfloat16, isOutput=False)
    wlm_p = nc.declare_dram_parameter("wlm", [NMC, P, DT, MC], dt.bfloat16, isOutput=False)
    bias_p = nc.declare_dram_parameter("bias_all", [L, P, 3 + NFC + DT + DT], dt.float32, isOutput=False)
    blm_p = nc.declare_dram_parameter("blm", [MC, NMC], dt.float32, isOutput=False)
    logits_p = nc.declare_dram_parameter("logits", [VPC, B * T], dt.float32, isOutput=True)

    RG = [list(range(NCORES))]

    with tile.TileContext(nc) as tc:
        with (
            tc.tile_pool(name="const", bufs=1) as cp,
            tc.tile_pool(name="wp", bufs=2) as wp,
            tc.tile_pool(name="ap", bufs=2) as app,
            tc.tile_pool(name="psA", bufs=3, space="PSUM") as psA,
            tc.tile_pool(name="psS", bufs=2, space="PSUM") as psS,
            tc.tile_pool(name="psY", bufs=3, space="PSUM") as psY,
            tc.tile_pool(name="dram", bufs=2, space="DRAM") as dramp,
        ):
            # ---------------- constants ----------------
            ident_bf = cp.tile([P, P], dt.bfloat16)
            make_identity(nc, ident_bf[:])
            ones_col_bf = cp.tile([P, 1], dt.bfloat16)
            nc.any.memset(ones_col_bf[:], 1.0)
            ones_row_f = cp.tile([1, P], dt.float32)
            nc.any.memset(ones_row_f[:], 1.0)
            ones_row_bf = cp.tile([1, P], dt.bfloat16)
            nc.any.memset(ones_row_bf[:], 1.0)
            eps_sb = cp.tile([1, 1], dt.float32)
            nc.any.memset(eps_sb[:], EPS)
            # causal 0/1 mask for S^T diagonal tiles: keep where q' >= k'
            cmask01 = cp.tile([P, P], dt.bfloat16)
            nc.gpsimd.memset(cmask01[:], 1.0)
            nc.gpsimd.affine_select(
                out=cmask01[:], in_=cmask01[:], compare_op=OP.is_ge,
                fill=0.0, base=0, pattern=[[1, P]], channel_multiplier=-1)

            # all per-layer biases in one tile [P, L, 61]
            NB = 3 + NFC + DT + DT
            bias_sb = cp.tile([P, L, NB], dt.float32)
            nc.sync.dma_start(bias_sb[:], bias_p[:].rearrange("l p c -> p l c"))

            def bqkvt(li):
                return bias_sb[:, li, 0:3]

            def bfc1t(li):
                return bias_sb[:, li, 3:3 + NFC]

            def bprojt(li):
                return bias_sb[:, li, 3 + NFC:3 + NFC + DT]

            def bfc2t(li):
                return bias_sb[:, li, 3 + NFC + DT:NB]

            blm_sb = cp.tile([MC, NMC], dt.float32)
            nc.sync.dma_start(blm_sb[:], blm_p[:])

            # tiny warm-up AllGather: absorbs cross-core kernel-start skew
            # while the embedding DMAs and first LN run
            if not no_coll:
                wu_sb = cp.tile([1, 16], dt.float32)
                nc.any.memset(wu_sb[:], 0.0)
                wu_in = dramp.tile([1, 16], dt.float32, name="wu_in", tag="wu_in",
                                   bufs=1)
                wu_out = dramp.tile([NCORES, 16], dt.float32, name="wu_out",
                                    tag="wu_out", bufs=1, addr_space="Shared")
                nc.sync.dma_start(wu_in[:], wu_sb[:])
                nc.gpsimd.collective_compute(
                    "AllGather", OP.bypass, replica_groups=RG,
                    ins=[wu_in[:].opt()], outs=[wu_out[:].opt()],
                )

            # ---------------- embedding ----------------
            hres = [cp.tile([P, DT, TPC], dt.float32, name=f"hres{h}") for h in range(B)]
            for half in range(B):
                nc.sync.dma_start(hres[half][:], emb_p[half])

            # ---------------- layernorm ----------------
            def ln_normalize(h_tile, name, out_ap=None):
                """Token-shard LN without gamma/beta (folded into weights).
                Returns bf16 [P, DT, TPC] tile of (h-m)*rstd (or writes into
                out_ap [P, DT, TPC] if given)."""
                hb = app.tile([P, DT, TPC], dt.bfloat16, name=f"hb_{name}", tag="hb")
                nc.vector.tensor_copy(hb[:], h_tile[:])
                hb2 = app.tile([P, DT, TPC], dt.bfloat16, name=f"hb2_{name}",
                               tag="hb2", bufs=2)
                nc.vector.tensor_mul(hb2[:], hb[:], hb[:])
                ps_sum = psY.tile([1, TPC], dt.float32, space="PSUM",
                                  name=f"psum_{name}", tag="small")
                ps_sq = psY.tile([1, TPC], dt.float32, space="PSUM",
                                 name=f"psq_{name}", tag="small")
                for dti in range(DT):
                    nc.tensor.matmul(ps_sum[:], lhsT=ones_col_bf[:], rhs=hb[:, dti, :],
                                     start=(dti == 0), stop=(dti == DT - 1))
                for dti in range(DT):
                    nc.tensor.matmul(ps_sq[:], lhsT=ones_col_bf[:], rhs=hb2[:, dti, :],
                                     start=(dti == 0), stop=(dti == DT - 1))
                m_sb = app.tile([1, TPC], dt.float32, name=f"m_{name}", tag="m")
                nc.vector.tensor_scalar_mul(m_sb[:], ps_sum[:], 1.0 / D)
                mm_sb = app.tile([1, TPC], dt.float32, name=f"mm_{name}", tag="mm")
                nc.vector.tensor_mul(mm_sb[:], m_sb[:], m_sb[:])
                var_sb = app.tile([1, TPC], dt.float32, name=f"var_{name}", tag="var")
                nc.vector.scalar_tensor_tensor(
                    out=var_sb[:], in0=ps_sq[:], scalar=1.0 / D, in1=mm_sb[:],
                    op0=OP.mult, op1=OP.subtract)
                std_sb = app.tile([1, TPC], dt.float32, name=f"std_{name}", tag="std")
                nc.scalar.activation(std_sb[:], var_sb[:], AF.Sqrt, bias=eps_sb[:])
                rstd_sb = app.tile([1, TPC], dt.float32, name=f"rstd_{name}", tag="rstd")
                nc.vector.reciprocal(rstd_sb[:], std_sb[:])
                mrstd_sb = app.tile([1, TPC], dt.float32, name=f"mrstd_{name}", tag="mrstd")
                nc.vector.scalar_tensor_tensor(
                    out=mrstd_sb[:], in0=m_sb[:], scalar=-1.0, in1=rstd_sb[:],
                    op0=OP.mult, op1=OP.mult)
                # broadcast across partitions via K=1 fp32 matmuls
                ps_r = psY.tile([P, TPC], dt.float32, space="PSUM",
                                name=f"psr_{name}", tag="small")
                nc.tensor.matmul(ps_r[:], lhsT=ones_row_f[:], rhs=rstd_sb[:],
                                 start=True, stop=True)
                rstd_full = app.tile([P, TPC], dt.bfloat16, name=f"rstdf_{name}", tag="rstdf")
                nc.vector.tensor_copy(rstd_full[:], ps_r[:])
                ps_mr = psY.tile([P, TPC], dt.float32, space="PSUM",
                                 name=f"psmr_{name}", tag="small")
                nc.tensor.matmul(ps_mr[:], lhsT=ones_row_f[:], rhs=mrstd_sb[:],
                                 start=True, stop=True)
                mrstd_full = app.tile([P, TPC], dt.bfloat16, name=f"mrstdf_{name}", tag="mrstdf")
                nc.vector.tensor_copy(mrstd_full[:], ps_mr[:])
                t1 = app.tile([P, DT, TPC], dt.bfloat16, name=f"t1_{name}",
                              tag="t1", bufs=2)
                nc.vector.tensor_tensor(
                    out=t1[:], in0=hb[:],
                    in1=rstd_full[:, None, :].to_broadcast([P, DT, TPC]), op=OP.mult)
                if out_ap is None:
                    hn = app.tile([P, DT, TPC], dt.bfloat16, name=f"hn_{name}",
                                  tag="hn", bufs=3)
                    out_ap = hn[:]
                else:
                    hn = None
                nc.vector.tensor_tensor(
                    out=out_ap, in0=t1[:],
                    in1=mrstd_full[:, None, :].to_broadcast([P, DT, TPC]), op=OP.add)
                return hn

            # ---------------- collectives ----------------
            def allgather_read(hn, name):
                """AllGather one half's LN'd shard; returns two aT tiles
                [P, 4, DT, TPC] bf16 (global token blocks 0-511 / 512-1023)
                so qkv can start as soon as the first block lands."""
                ag_in = dramp.tile([P * DT, TPC], dt.bfloat16,
                                   name=f"agin_{name}", tag="agin")
                nc.sync.dma_start(
                    ag_in[:].rearrange("(p dt) t -> p dt t", p=P), hn[:])
                ag_out = dramp.tile([NCORES * P * DT, TPC], dt.bfloat16,
                                    name=f"agout_{name}", tag="agout",
                                    addr_space="Shared")
                if no_coll:
                    nc.sync.dma_start(ag_out[0:P * DT, :], ag_in[:])
                else:
                    nc.gpsimd.collective_compute(
                        "AllGather", OP.bypass, replica_groups=RG,
                        ins=[ag_in[:].opt()], outs=[ag_out[:].opt()],
                    )
                ag_view = ag_out[:].rearrange(
                    "(rb r p dt) t -> rb p r (dt t)", rb=2, p=P, dt=DT)
                aTs = []
                for rb in range(2):
                    aT = app.tile([P, 4, DT * TPC], dt.bfloat16,
                                  name=f"aT_{name}{rb}", tag="aT", bufs=4)
                    nc.sync.dma_start(aT[:], ag_view[rb])
                    aTs.append(aT.rearrange("p r (dt t) -> p r dt t", dt=DT))
                return aTs

            def reduce_scatter_residual(rs_in, li, h_tile, name):
                """rs_in: DRAM [NCORES, P, DT, TPC] bf16 feature-major partials
                blocked by destination core. RS, add partial+bias into residual."""
                rs_out = dramp.tile([P * DT, TPC], dt.bfloat16, name=f"rsout_{name}",
                                    tag="rsout")
                if no_coll:
                    nc.sync.dma_start(
                        rs_out[:], rs_in[0].rearrange("p dt t -> (p dt) t"))
                else:
                    nc.gpsimd.collective_compute(
                        "ReduceScatter", OP.add, replica_groups=RG,
                        ins=[rs_in[:].rearrange("r p dt t -> (r p dt) t").opt()],
                        outs=[rs_out[:].opt()],
                    )
                rsb = app.tile([P, DT, TPC], dt.bfloat16, name=f"rsb_{name}", tag="rsb")
                nc.sync.dma_start(rsb[:], rs_out[:].rearrange("(p dc) t -> p dc t", p=P))
                bias_t = bprojt(li)
                for dc in range(DT):
                    nc.vector.scalar_tensor_tensor(
                        out=h_tile[:, dc, :], in0=rsb[:, dc, :],
                        scalar=bias_t[:, dc:dc + 1], in1=h_tile[:, dc, :],
                        op0=OP.add, op1=OP.add)

            # ---------------- layer blocks ----------------
            def load_weights(li):
                wqkv = wp.tile([P, DT, QKVC], dt.bfloat16, name=f"wqkv{li}", tag="wqkv")
                nc.sync.dma_start(wqkv[:], wqkv_p[li])
                wproj = wp.tile([P, D], dt.bfloat16, name=f"wproj{li}", tag="wproj")
                nc.sync.dma_start(wproj[:], wproj_p[li])
                return wqkv, wproj

            def qkv_block(aT2, wqkv, li, half):
                """q,k,v feature-major [P, 3, T] bf16 (+bias)."""
                qkT = app.tile([P, 3, T], dt.bfloat16, name=f"qkT{half}", tag="qkT")
                bq = bqkvt(li)
                for tk in range(NT):
                    for c in range(3):
                        ps = psA.tile([P, 512], dt.float32, space="PSUM",
                                      name="ps_qkv", tag="psA")
                        for dti in range(DT):
                            nc.tensor.matmul(
                                ps[:], lhsT=wqkv[:, dti, c * P:(c + 1) * P],
                                rhs=aT2[tk][:, :, dti, :],
                                start=(dti == 0), stop=(dti == DT - 1))
                        if c % 2 == 0:
                            nc.vector.tensor_scalar_add(
                                qkT[:, c, tk * 512:(tk + 1) * 512], ps[:],
                                bq[:, c:c + 1])
                        else:
                            nc.scalar.add(
                                qkT[:, c, tk * 512:(tk + 1) * 512], ps[:],
                                bq[:, c:c + 1])
                return qkT

            def attention(qkT, half):
                # v -> token-major [128(t), 8(tt), 2(h2), 65] with a ones col
                v_tok = app.tile([P, DT, HPC, HD + 1], dt.bfloat16,
                                 name=f"vtok{half}", tag="vtok")
                nc.vector.memset(v_tok[:, :, :, HD:HD + 1], 1.0)
                for tt in range(DT):
                    pst = psY.tile([P, P], dt.bfloat16, space="PSUM", name="pst_v",
                                   tag="small")
                    nc.tensor.transpose(
                        pst[:], qkT[:, 2, tt * P:(tt + 1) * P], ident_bf[:])
                    nc.vector.tensor_copy(
                        v_tok[:, tt, :, 0:HD],
                        pst[:].rearrange("p (h2 d) -> p h2 d", h2=HPC))

                y_sb = app.tile([P, DT, P], dt.bfloat16, name=f"ysb{half}", tag="ysb")
                yT = app.tile([P, T], dt.bfloat16, name=f"yT{half}", tag="yT")
                for qc in range(NT):  # 512-query chunks
                    PTs = [app.tile([P, 8, 512], dt.bfloat16,
                                    name=f"PT{half}_{qc}_{h2}", tag=f"PT{h2}",
                                    bufs=1)
                           for h2 in range(HPC)]
                    for h2 in range(HPC):
                        hs = h2 * HD
                        nkt = qc * 4 + 4
                        for kt in range(nkt):
                            off = max(0, kt * P - qc * 512)
                            ps_st = psS.tile([P, 512], dt.float32, space="PSUM",
                                             name="ps_st", tag="psS")
                            nc.tensor.matmul(
                                ps_st[:, off:512],
                                lhsT=qkT[hs:hs + HD, 1, kt * P:(kt + 1) * P],
                                rhs=qkT[hs:hs + HD, 0,
                                        qc * 512 + off:(qc + 1) * 512],
                                start=True, stop=True)
                            nc.scalar.activation(
                                PTs[h2][:, kt, off:512], ps_st[:, off:512],
                                AF.Exp, scale=ATT_SCALE)
                            if kt >= qc * 4:
                                # causal zeroing of the diagonal 128-sub-tile
                                # (on DVE: the gpsimd queue carries collective
                                # waits and must stay clear)
                                nc.vector.tensor_mul(
                                    PTs[h2][:, kt, off:off + P],
                                    PTs[h2][:, kt, off:off + P],
                                    cmask01[:])
                    # AV token-major, fused row-sum via the ones column
                    for qt in range(qc * 4, qc * 4 + 4):
                        qoff = qt * P - qc * 512
                        for h2 in range(HPC):
                            hs = h2 * HD
                            ps_y = psY.tile([P, HD + 1], dt.float32, space="PSUM",
                                            name="ps_y", tag="small")
                            for kt in range(qt + 1):
                                nc.tensor.matmul(
                                    ps_y[:], lhsT=PTs[h2][:, kt, qoff:qoff + P],
                                    rhs=v_tok[:, kt, h2, :],
                                    start=(kt == 0), stop=(kt == qt))
                            rec = app.tile([P, 1], dt.float32, name="rec", tag="rec",
                                           bufs=3)
                            nc.vector.reciprocal(rec[:], ps_y[:, HD:HD + 1])
                            nc.vector.tensor_scalar_mul(
                                y_sb[:, qt, hs:hs + HD], ps_y[:, 0:HD], rec[:, 0:1])
                    for qt in range(qc * 4, qc * 4 + 4):
                        ps_t = psY.tile([P, P], dt.bfloat16, space="PSUM",
                                        name="ps_t", tag="small")
                        nc.tensor.transpose(ps_t[:], y_sb[:, qt, :], ident_bf[:])
                        nc.vector.tensor_copy(yT[:, qt * P:(qt + 1) * P], ps_t[:])
                return yT

            def proj_partial(yT, wproj, half, name):
                """Partial attn output, feature-major, blocked by destination
                core: rs_in [NCORES, P, DT, TPC] bf16."""
                rs_in = dramp.tile([NCORES, P, DT, TPC], dt.bfloat16,
                                   name=f"rsin_{name}", tag="rsin")
                for tk in range(NT):
                    prd = app.tile([P, DT, 512], dt.bfloat16, name=f"prd{half}_{tk}",
                                   tag="prd")
                    for dc in range(DT):
                        ps = psA.tile([P, 512], dt.float32, space="PSUM",
                                      name="ps_pr", tag="psA")
                        nc.tensor.matmul(
                            ps[:], lhsT=wproj[:, dc * P:(dc + 1) * P],
                            rhs=yT[:, tk * 512:(tk + 1) * 512], start=True, stop=True)
                        if dc % 2 == 0:
                            nc.vector.tensor_copy(prd[:, dc, :], ps[:])
                        else:
                            nc.scalar.copy(prd[:, dc, :], ps[:])
                    for tb in range(tk * 4, (tk + 1) * 4):
                        nc.sync.dma_start(
                            rs_in[tb], prd[:, :, (tb - tk * 4) * P:(tb - tk * 4 + 1) * P])
                return rs_in

            def mlp_half(hn2, li, half):
                """Token-local MLP for one half (128 tokens) with streamed
                bf16 fc weights. Run per half so one half's MLP hides the
                other half's collectives."""
                mTm = app.tile([P, NFC, TPC], dt.bfloat16, name=f"mTm{li}_{half}",
                               tag="mTm")
                b1 = bfc1t(li)
                for fc in range(NFC):
                    wf1c = wp.tile([P, DT, P], dt.bfloat16,
                                   name=f"wf1c{li}_{half}_{fc}", tag="wf1c",
                                   bufs=4)
                    nc.sync.dma_start(wf1c[:], wfc1_p[li, fc])
                    ps = psA.tile([P, TPC], dt.float32, space="PSUM",
                                  name="ps_f1", tag="psA")
                    for dti in range(DT):
                        nc.tensor.matmul(
                            ps[:], lhsT=wf1c[:, dti, :], rhs=hn2[:, dti, :],
                            start=(dti == 0), stop=(dti == DT - 1))
                    nc.scalar.activation(
                        mTm[:, fc, :], ps[:], AF.Gelu, bias=b1[:, fc:fc + 1])
                b2 = bfc2t(li)
                for dc in range(DT):
                    wf2c = wp.tile([P, NFC, P], dt.bfloat16,
                                   name=f"wf2c{li}_{half}_{dc}", tag="wf2c",
                                   bufs=2)
                    nc.sync.dma_start(wf2c[:], wfc2_p[li, dc])
                    ps2 = psA.tile([P, TPC], dt.float32, space="PSUM",
                                   name="ps_f2", tag="psA")
                    for kt in range(NFC):
                        nc.tensor.matmul(
                            ps2[:], lhsT=wf2c[:, kt, :], rhs=mTm[:, kt, :],
                            start=(kt == 0), stop=(kt == NFC - 1))
                    nc.vector.scalar_tensor_tensor(
                        out=hres[half][:, dc, :], in0=ps2[:],
                        scalar=b2[:, dc:dc + 1],
                        in1=hres[half][:, dc, :], op0=OP.add, op1=OP.add)

            # ---------------- transformer layers ----------------
            # the two batch halves run as independent software-pipelined
            # streams: while one half waits on its AllGather/ReduceScatter,
            # the other half's attention/MLP keeps the PE busy
            ag = []
            for h in range(B):
                hn0 = ln_normalize(hres[h], f"pro{h}")
                ag.append(allgather_read(hn0, f"l0a{h}"))
            for li in range(L):
                wqkv, wproj = load_weights(li)
                ag_next = [None, None]
                for h in range(B):
                    qkT = qkv_block(ag[h], wqkv, li, h)
                    yT = attention(qkT, h)
                    rs_in = proj_partial(yT, wproj, h, f"l{li}p{h}")
                    reduce_scatter_residual(rs_in, li, hres[h], f"l{li}p{h}")
                    hn2 = ln_normalize(hres[h], f"l{li}m{h}")
                    mlp_half(hn2, li, h)
                    # trailing LN + AllGather feeds the next layer (or, after
                    # the last layer, the folded final LN for the LM head)
                    hn1 = ln_normalize(hres[h], f"l{li}n{h}")
                    ag_next[h] = allgather_read(hn1, f"l{li + 1}a{h}")
                ag = ag_next

            # ---------------- LM head ----------------
            afTs = ag
            for mc in range(NMC):
                wlm = wp.tile([P, DT, MC], dt.bfloat16, name=f"wlm{mc}", tag="wlm",
                              bufs=3)
                nc.sync.dma_start(wlm[:], wlm_p[mc])
                # per pass: two 512-token chunks accumulate in two PSUM banks;
                # consecutive matmuls share the same stationary wlm slice so
                # the LDWEIGHTS pipeline stays fed
                for hp in range(B):
                    psL = [psA.tile([MC, 512], dt.float32, space="PSUM",
                                    name=f"ps_lm{mc}_{hp}_{i}", tag="psA")
                           for i in range(2)]
                    for dti in range(DT):
                        for tq in range(2):
                            nc.tensor.matmul(
                                psL[tq][:],
                                lhsT=wlm[:, dti, :],
                                rhs=afTs[hp][tq][:, :, dti, :],
                                start=(dti == 0), stop=(dti == DT - 1))
                    for tq in range(2):
                        lsb = app.tile([MC, 512], dt.float32, name="lsb",
                                       tag="lsb", bufs=4)
                        if tq == 0:
                            nc.vector.tensor_scalar_add(
                                lsb[:], psL[tq][:], blm_sb[:, mc:mc + 1])
                        else:
                            nc.scalar.add(lsb[:], psL[tq][:], blm_sb[:, mc:mc + 1])
                        nc.gpsimd.dma_start(
                            logits_p[mc * MC:(mc + 1) * MC,
                                     hp * T + tq * 512:hp * T + (tq + 1) * 512],
                            lsb[:])

    nc.compile()
    return nc


def _get_nc():
    no_coll = os.environ.get("KERNEL_NO_COLL", "0") == "1"
    key = ("nc", no_coll)
    if key not in _CACHE:
        _CACHE[key] = _build_nc(no_coll)
    return _CACHE[key]


def build_in_maps(input_ids, wte, wpe, ln1_g, ln1_b, w_qkv, b_qkv, w_proj,
                  b_proj, ln2_g, ln2_b, w_fc1, b_fc1, w_fc2, b_fc2, lnf_g,
                  lnf_b, w_lm):
    import ml_dtypes
    f32 = np.float32
    bf16 = ml_dtypes.bfloat16

    ids = np.asarray(input_ids).astype(np.int64)
    wte = np.asarray(wte, dtype=f32)
    wpe = np.asarray(wpe, dtype=f32)
    g1 = np.asarray(ln1_g, f32)
    b1 = np.asarray(ln1_b, f32)
    g2 = np.asarray(ln2_g, f32)
    b2 = np.asarray(ln2_b, f32)
    gf = np.asarray(lnf_g, f32)
    bf = np.asarray(lnf_b, f32)
    Wq = np.asarray(w_qkv, f32)
    Wp = np.asarray(w_proj, f32)
    W1 = np.asarray(w_fc1, f32)
    W2 = np.asarray(w_fc2, f32)
    Wlm = np.asarray(w_lm, f32)
    bq = np.asarray(b_qkv, f32)
    bp = np.asarray(b_proj, f32)
    bb1 = np.asarray(b_fc1, f32)
    bb2 = np.asarray(b_fc2, f32)

    # fold LN gains into consuming weights; betas into their biases
    Wq_f = Wq * g1[:, :, None]                       # [L, D, 3D]
    bq_f = np.einsum('ld,ldo->lo', b1, Wq) + bq      # [L, 3D]
    W1_f = W1 * g2[:, :, None]                       # [L, D, 4D]
    b1_f = np.einsum('ld,ldo->lo', b2, W1) + bb1     # [L, 4D]
    Wlm_f = Wlm * gf[:, None]                        # [D, V]
    blm_f = bf @ Wlm                                 # [V]

    # embeddings, feature-major per core
    emb = wte[ids] + wpe[None, :, :]                 # [B, T, D]

    in_maps = []
    for r in range(NCORES):
        t0, t1 = r * TPC, (r + 1) * TPC
        cols = np.r_[P * r:P * r + P, D + P * r:D + P * r + P,
                     2 * D + P * r:2 * D + P * r + P]
        vs, ve = r * VPC, (r + 1) * VPC

        # emb_fm [B, P, DT, TPC]
        e = emb[:, t0:t1, :]                         # [B, TPC, D]
        emb_fm = np.ascontiguousarray(
            e.transpose(0, 2, 1).reshape(B, DT, P, TPC).transpose(0, 2, 1, 3))

        # wqkv [L, P, DT, QKVC]
        wq = Wq_f[:, :, cols]                        # [L, D, 384]
        wq = wq.reshape(L, DT, P, QKVC).transpose(0, 2, 1, 3)

        # wproj [L, P, D] (rows P*r..P*r+P)
        wpj = Wp[:, P * r:P * r + P, :]

        # wfc1 [L, NFC, P, DT, P]
        w1 = W1_f.reshape(L, DT, P, NFC, P).transpose(0, 3, 2, 1, 4)

        # wfc2 [L, DT, P, NFC, P]
        w2 = W2.reshape(L, NFC, P, DT, P).transpose(0, 3, 2, 1, 4)

        # wlm [NMC, P, DT, MC]
        wl = Wlm_f[:, vs:ve].reshape(DT, P, NMC, MC).transpose(2, 1, 0, 3)

        # bias_all [L, P, 3 + NFC + DT + DT]
        bias_all = np.concatenate([
            bq_f[:, cols].reshape(L, 3, P).transpose(0, 2, 1),
            b1_f.reshape(L, NFC, P).transpose(0, 2, 1),
            bp.reshape(L, DT, P).transpose(0, 2, 1),
            bb2.reshape(L, DT, P).transpose(0, 2, 1),
        ], axis=2)

        m = {
            "emb_fm": emb_fm,
            "wqkv": np.ascontiguousarray(wq.astype(bf16)),
            "wproj": np.ascontiguousarray(wpj.astype(bf16)),
            "wfc1": np.ascontiguousarray(w1.astype(bf16)),
            "wfc2": np.ascontiguousarray(w2.astype(bf16)),
            "wlm": np.ascontiguousarray(wl.astype(bf16)),
            "bias_all": np.ascontiguousarray(bias_all),
            "blm": np.ascontiguousarray(
                blm_f[vs:ve].reshape(NMC, MC).T.astype(f32)),
        }
        in_maps.append(m)

    return in_maps


def kernel(**inputs):
    global last_exec_time_ns
    from concourse.bass_utils import run_bass_kernel_spmd

    in_maps = build_in_maps(**inputs)
    nc = _get_nc()
    trace = os.environ.get("KERNEL_TRACE", "0") == "1"
    res = run_bass_kernel_spmd(nc, in_maps, list(range(NCORES)), trace=trace)
    last_exec_time_ns = res.exec_time_ns

    parts = [res.results[r]["logits"] for r in range(NCORES)]  # [VPC, B*T] each
    full = np.concatenate(parts, axis=0)          # [V, B*T]
    out = full.T.reshape(B, T, V).astype(np.float32)
    return out


# revision 8
# speedup vs baseline: 1.1488x; 1.1383x over previous
"""Trainium2 Bass kernel for nn_LuminaLM (4-layer GPT-2-like transformer + LM head).

Strategy: 8-way Megatron tensor parallel with sequence-parallel residual.
 - Host precomputes embeddings (feature-major), folds LN gamma/beta into the
   consuming weights, casts all weights to bf16, and pre-transposes layouts so
   every DMA is contiguous per partition.
 - Each core owns 2 of 16 heads, 1/8 of the vocab; MLP is token-local
   (full fc weights streamed bf16) over the core's 256 tokens.
 - Residual h is token-sharded feature-major [128(dp), 8(dt), 128(t)] fp32.
 - Per layer-half: LN stats via ones-matmuls, normalize, AllGather bf16,
   qkv -> attention -> proj partial -> ReduceScatter bf16 -> residual add.
 - Attention: S^T computed directly per k-tile (k stationary, wide-N ragged),
   exp on ScalarE straight into SBUF P^T, causal zeroing via gpsimd
   affine_select, AV token-major with a fused ones-column giving row sums,
   softmax normalization via per-partition reciprocal, PE transpose to y^T.
 - LM head: activation-stationary (LDWEIGHTS amortized), vocab-sharded,
   token-major logits written fp32 via gpsimd-issued DMAs.
Matmuls are bf16 with fp32 PSUM accumulation; collectives ride bf16.
"""

import os
import numpy as np

B, T, D, V, L = 2, 1024, 1024, 32000, 4
H, HD = 16, 64
NCORES = 8
P = 128
TPC = T // NCORES          # 128 tokens per core per batch
HPC = H // NCORES          # 2 heads per core
QKVC = 3 * P               # 384 qkv cols per core (q:128, k:128, v:128)
VPC = V // NCORES          # 4000 vocab per core
MC = 125                   # lm-head vocab chunk (32 chunks of 125 = 4000)
NMC = VPC // MC            # 32
DT = D // P                # 8 d-tiles
NFC = 4 * D // P           # 32 fc1-output chunks
NT = T // 512              # 2 token chunks of 512 per half
EPS = 1e-5
ATT_SCALE = 1.0 / np.sqrt(HD)

_CACHE = {}
last_exec_time_ns = None


def _build_nc(no_coll=False):
    import concourse.bass as bass
    import concourse.mybir as mybir
    import concourse.tile as tile
    from concourse import bacc
    from concourse.masks import make_identity

    dt = mybir.dt
    AF = mybir.ActivationFunctionType
    OP = mybir.AluOpType

    nc = bacc.Bacc("TRN2", target_bir_lowering=False, debug=False,
                   num_devices=NCORES)

    # ---- external parameters (per-core shards, staged by host) ----
    emb_p = nc.declare_dram_parameter("emb_fm", [B, P, DT, TPC], dt.float32, isOutput=False)
    wqkv_p = nc.declare_dram_parameter("wqkv", [L, P, DT, QKVC], dt.bfloat16, isOutput=False)
    wproj_p = nc.declare_dram_parameter("wproj", [L, P, D], dt.bfloat16, isOutput=False)
    wfc1_p = nc.declare_dram_parameter("wfc1", [L, NFC, P, DT, P], dt.bfloat16, isOutput=False)
    wfc2_p = nc.declare_dram_parameter("wfc2", [L, DT, P, NFC, P], dt.bfloat16, isOutput=False)
    wlm_p = nc.declare_dram_parameter("wlm", [NMC, P, DT, MC], dt.bfloat16, isOutput=False)
    bias_p = nc.declare_dram_parameter("bias_all", [L, P, 3 + NFC + DT + DT], dt.float32, isOutput=False)
    blm_p = nc.declare_dram_parameter("blm", [MC, NMC], dt.float32, isOutput=False)
    logits_p = nc.declare_dram_parameter("logits", [VPC, B * T], dt.float32, isOutput=True)

    RG = [list(range(NCORES))]

    with tile.TileContext(nc) as tc:
        with (
            tc.tile_pool(name="const", bufs=1) as cp,
            tc.tile_pool(name="wp", bufs=2) as wp,
            tc.tile_pool(name="ap", bufs=2) as app,
            tc.tile_pool(name="psA", bufs=3, space="PSUM") as psA,
            tc.tile_pool(name="psS", bufs=2, space="PSUM") as psS,
            tc.tile_pool(name="psY", bufs=3, space="PSUM") as psY,
            tc.tile_pool(name="dram", bufs=2, space="DRAM") as dramp,
        ):
            # ---------------- constants ----------------
            ident_bf = cp.tile([P, P], dt.bfloat16)
            make_identity(nc, ident_bf[:])
            ones_col_bf = cp.tile([P, 1], dt.bfloat16)
            nc.any.memset(ones_col_bf[:], 1.0)
            ones_row_f = cp.tile([1, P], dt.float32)
            nc.any.memset(ones_row_f[:], 1.0)
            ones_row_bf = cp.tile([1, P], dt.bfloat16)
            nc.any.memset(ones_row_bf[:], 1.0)
            eps_sb = cp.tile([1, 1], dt.float32)
            nc.any.memset(eps_sb[:], EPS)
            # causal 0/1 mask for S^T diagonal tiles: keep where q' >= k'
            cmask01 = cp.tile([P, P], dt.bfloat16)
            nc.gpsimd.memset(cmask01[:], 1.0)
            nc.gpsimd.affine_select(
                out=cmask01[:], in_=cmask01[:], compare_op=OP.is_ge,
                fill=0.0, base=0, pattern=[[1, P]], channel_multiplier=-1)

            # all per-layer biases in one tile [P, L, 61]
            NB = 3 + NFC + DT + DT
            bias_sb = cp.tile([P, L, NB], dt.float32)
            nc.sync.dma_start(bias_sb[:], bias_p[:].rearrange("l p c -> p l c"))

            def bqkvt(li):
                return bias_sb[:, li, 0:3]

            def bfc1t(li):
                return bias_sb[:, li, 3:3 + NFC]

            def bprojt(li):
                return bias_sb[:, li, 3 + NFC:3 + NFC + DT]

            def bfc2t(li):
                return bias_sb[:, li, 3 + NFC + DT:NB]

            blm_sb = cp.tile([MC, NMC], dt.float32)
            nc.sync.dma_start(blm_sb[:], blm_p[:])

            # tiny warm-up AllGather: absorbs cross-core kernel-start skew
            # while the embedding DMAs and first LN run
            if not no_coll:
                wu_sb = cp.tile([1, 16], dt.float32)
                nc.any.memset(wu_sb[:], 0.0)
                wu_in = dramp.tile([1, 16], dt.float32, name="wu_in", tag="wu_in",
                                   bufs=1)
                wu_out = dramp.tile([NCORES, 16], dt.float32, name="wu_out",
                                    tag="wu_out", bufs=1, addr_space="Shared")
                nc.sync.dma_start(wu_in[:], wu_sb[:])
                nc.gpsimd.collective_compute(
                    "AllGather", OP.bypass, replica_groups=RG,
                    ins=[wu_in[:].opt()], outs=[wu_out[:].opt()],
                )

            # ---------------- embedding ----------------
            hres = [cp.tile([P, DT, TPC], dt.float32, name=f"hres{h}") for h in range(B)]
            for half in range(B):
                nc.sync.dma_start(hres[half][:], emb_p[half])

            # ---------------- layernorm ----------------
            def ln_normalize(h_tile, name, out_ap=None):
                """Token-shard LN without gamma/beta (folded into weights).
                Returns bf16 [P, DT, TPC] tile of (h-m)*rstd (or writes into
                out_ap [P, DT, TPC] if given)."""
                hb = app.tile([P, DT, TPC], dt.bfloat16, name=f"hb_{name}", tag="hb")
                nc.vector.tensor_copy(hb[:], h_tile[:])
                hb2 = app.tile([P, DT, TPC], dt.bfloat16, name=f"hb2_{name}",
                               tag="hb2", bufs=2)
                nc.vector.tensor_mul(hb2[:], hb[:], hb[:])
                ps_sum = psY.tile([1, TPC], dt.float32, space="PSUM",
                                  name=f"psum_{name}", tag="small")
                ps_sq = psY.tile([1, TPC], dt.float32, space="PSUM",
                                 name=f"psq_{name}", tag="small")
                for dti in range(DT):
                    nc.tensor.matmul(ps_sum[:], lhsT=ones_col_bf[:], rhs=hb[:, dti, :],
                                     start=(dti == 0), stop=(dti == DT - 1))
                for dti in range(DT):
                    nc.tensor.matmul(ps_sq[:], lhsT=ones_col_bf[:], rhs=hb2[:, dti, :],
                                     start=(dti == 0), stop=(dti == DT - 1))
                m_sb = app.tile([1, TPC], dt.float32, name=f"m_{name}", tag="m")
                nc.vector.tensor_scalar_mul(m_sb[:], ps_sum[:], 1.0 / D)
                mm_sb = app.tile([1, TPC], dt.float32, name=f"mm_{name}", tag="mm")
                nc.vector.tensor_mul(mm_sb[:], m_sb[:], m_sb[:])
                var_sb = app.tile([1, TPC], dt.float32, name=f"var_{name}", tag="var")
                nc.vector.scalar_tensor_tensor(
                    out=var_sb[:], in0=ps_sq[:], scalar=1.0 / D, in1=mm_sb[:],
                    op0=OP.mult, op1=OP.subtract)
                std_sb = app.tile([1, TPC], dt.float32, name=f"std_{name}", tag="std")
                nc.scalar.activation(std_sb[:], var_sb[:], AF.Sqrt, bias=eps_sb[:])
                rstd_sb = app.tile([1, TPC], dt.float32, name=f"rstd_{name}", tag="rstd")
                nc.vector.reciprocal(rstd_sb[:], std_sb[:])
                mrstd_sb = app.tile([1, TPC], dt.float32, name=f"mrstd_{name}", tag="mrstd")
                nc.vector.scalar_tensor_tensor(
                    out=mrstd_sb[:], in0=m_sb[:], scalar=-1.0, in1=rstd_sb[:],
                    op0=OP.mult, op1=OP.mult)
                # broadcast across partitions via K=1 fp32 matmuls
                ps_r = psY.tile([P, TPC], dt.float32, space="PSUM",
                                name=f"psr_{name}", tag="small")
                nc.tensor.matmul(ps_r[:], lhsT=ones_row_f[:], rhs=rstd_sb[:],
                                 start=True, stop=True)
                rstd_full = app.tile([P, TPC], dt.bfloat16, name=f"rstdf_{name}", tag="rstdf")
                nc.vector.tensor_copy(rstd_full[:], ps_r[:])
                ps_mr = psY.tile([P, TPC], dt.float32, space="PSUM",
                                 name=f"psmr_{name}", tag="small")
                nc.tensor.matmul(ps_mr[:], lhsT=ones_row_f[:], rhs=mrstd_sb[:],
                                 start=True, stop=True)
                mrstd_full = app.tile([P, TPC], dt.bfloat16, name=f"mrstdf_{name}", tag="mrstdf")
                nc.vector.tensor_copy(mrstd_full[:], ps_mr[:])
                t1 = app.tile([P, DT, TPC], dt.bfloat16, name=f"t1_{name}",
                              tag="t1", bufs=2)
                nc.vector.tensor_tensor(
                    out=t1[:], in0=hb[:],
                    in1=rstd_full[:, None, :].to_broadcast([P, DT, TPC]), op=OP.mult)
                if out_ap is None:
                    hn = app.tile([P, DT, TPC], dt.bfloat16, name=f"hn_{name}",
                                  tag="hn", bufs=3)
                    out_ap = hn[:]
                else:
                    hn = None
                nc.vector.tensor_tensor(
                    out=out_ap, in0=t1[:],
                    in1=mrstd_full[:, None, :].to_broadcast([P, DT, TPC]), op=OP.add)
                return hn

            # ---------------- collectives ----------------
            def allgather_read(hn, name):
                """AllGather one half's LN'd shard; returns two aT tiles
                [P, 4, DT, TPC] bf16 (global token blocks 0-511 / 512-1023)
                so qkv can start as soon as the first block lands."""
                ag_in = dramp.tile([P * DT, TPC], dt.bfloat16,
                                   name=f"agin_{name}", tag="agin")
                nc.sync.dma_start(
                    ag_in[:].rearrange("(p dt) t -> p dt t", p=P), hn[:])
                ag_out = dramp.tile([NCORES * P * DT, TPC], dt.bfloat16,
                                    name=f"agout_{name}", tag="agout",
                                    addr_space="Shared")
                if no_coll:
                    nc.sync.dma_start(ag_out[0:P * DT, :], ag_in[:])
                else:
                    nc.gpsimd.collective_compute(
                        "AllGather", OP.bypass, replica_groups=RG,
                        ins=[ag_in[:].opt()], outs=[ag_out[:].opt()],
                    )
                ag_view = ag_out[:].rearrange(
                    "(rb r p dt) t -> rb p r (dt t)", rb=2, p=P, dt=DT)
                aTs = []
                for rb in range(2):
                    aT = app.tile([P, 4, DT * TPC], dt.bfloat16,
                                  name=f"aT_{name}{rb}", tag="aT", bufs=4)
                    nc.sync.dma_start(aT[:], ag_view[rb])
                    aTs.append(aT.rearrange("p r (dt t) -> p r dt t", dt=DT))
                return aTs

            def rs_start(rs_in, name):
                """Trigger the ReduceScatter for rs_in [NCORES, P, DT, TPC]
                bf16 feature-major partials blocked by destination core."""
                rs_out = dramp.tile([P * DT, TPC], dt.bfloat16, name=f"rsout_{name}",
                                    tag="rsout")
                if no_coll:
                    nc.sync.dma_start(
                        rs_out[:], rs_in[0].rearrange("p dt t -> (p dt) t"))
                else:
                    nc.gpsimd.collective_compute(
                        "ReduceScatter", OP.add, replica_groups=RG,
                        ins=[rs_in[:].rearrange("r p dt t -> (r p dt) t").opt()],
                        outs=[rs_out[:].opt()],
                    )
                return rs_out

            def rs_finish(rs_out, li, h_tile, name):
                """Read back the RS result, add partial+bias into residual."""
                rsb = app.tile([P, DT, TPC], dt.bfloat16, name=f"rsb_{name}", tag="rsb")
                nc.scalar.dma_start(rsb[:], rs_out[:].rearrange("(p dc) t -> p dc t", p=P))
                bias_t = bprojt(li)
                for dc in range(DT):
                    nc.vector.scalar_tensor_tensor(
                        out=h_tile[:, dc, :], in0=rsb[:, dc, :],
                        scalar=bias_t[:, dc:dc + 1], in1=h_tile[:, dc, :],
                        op0=OP.add, op1=OP.add)

            # ---------------- layer blocks ----------------
            def load_weights(li):
                wqkv = wp.tile([P, DT, QKVC], dt.bfloat16, name=f"wqkv{li}", tag="wqkv")
                nc.sync.dma_start(wqkv[:], wqkv_p[li])
                wproj = wp.tile([P, D], dt.bfloat16, name=f"wproj{li}", tag="wproj")
                nc.sync.dma_start(wproj[:], wproj_p[li])
                return wqkv, wproj

            def qkv_block(aT2, wqkv, li, half):
                """q,k,v feature-major [P, 3, T] bf16 (+bias)."""
                qkT = app.tile([P, 3, T], dt.bfloat16, name=f"qkT{half}", tag="qkT")
                bq = bqkvt(li)
                for tk in range(NT):
                    for c in range(3):
                        ps = psA.tile([P, 512], dt.float32, space="PSUM",
                                      name="ps_qkv", tag="psA")
                        for dti in range(DT):
                            nc.tensor.matmul(
                                ps[:], lhsT=wqkv[:, dti, c * P:(c + 1) * P],
                                rhs=aT2[tk][:, :, dti, :],
                                start=(dti == 0), stop=(dti == DT - 1))
                        if c % 2 == 0:
                            nc.vector.tensor_scalar_add(
                                qkT[:, c, tk * 512:(tk + 1) * 512], ps[:],
                                bq[:, c:c + 1])
                        else:
                            nc.scalar.add(
                                qkT[:, c, tk * 512:(tk + 1) * 512], ps[:],
                                bq[:, c:c + 1])
                return qkT

            def attention(qkT, half):
                # v -> token-major [128(t), 8(tt), 2(h2), 65] with a ones col
                v_tok = app.tile([P, DT, HPC, HD + 1], dt.bfloat16,
                                 name=f"vtok{half}", tag="vtok")
                nc.vector.memset(v_tok[:, :, :, HD:HD + 1], 1.0)
                for tt in range(DT):
                    pst = psY.tile([P, P], dt.bfloat16, space="PSUM", name="pst_v",
                                   tag="small")
                    nc.tensor.transpose(
                        pst[:], qkT[:, 2, tt * P:(tt + 1) * P], ident_bf[:])
                    nc.vector.tensor_copy(
                        v_tok[:, tt, :, 0:HD],
                        pst[:].rearrange("p (h2 d) -> p h2 d", h2=HPC))

                y_sb = app.tile([P, DT, P], dt.bfloat16, name=f"ysb{half}", tag="ysb")
                yT = app.tile([P, T], dt.bfloat16, name=f"yT{half}", tag="yT")
                for qc in range(NT):  # 512-query chunks
                    PTs = [app.tile([P, 8, 512], dt.bfloat16,
                                    name=f"PT{half}_{qc}_{h2}", tag=f"PT{h2}",
                                    bufs=1)
                           for h2 in range(HPC)]
                    for h2 in range(HPC):
                        hs = h2 * HD
                        nkt = qc * 4 + 4
                        for kt in range(nkt):
                            off = max(0, kt * P - qc * 512)
                            ps_st = psS.tile([P, 512], dt.float32, space="PSUM",
                                             name="ps_st", tag="psS")
                            nc.tensor.matmul(
                                ps_st[:, off:512],
                                lhsT=qkT[hs:hs + HD, 1, kt * P:(kt + 1) * P],
                                rhs=qkT[hs:hs + HD, 0,
                                        qc * 512 + off:(qc + 1) * 512],
                                start=True, stop=True)
                            nc.scalar.activation(
                                PTs[h2][:, kt, off:512], ps_st[:, off:512],
                                AF.Exp, scale=ATT_SCALE)
                            if kt >= qc * 4:
                                # causal zeroing of the diagonal 128-sub-tile
                                # (on DVE: the gpsimd queue carries collective
                                # waits and must stay clear)
                                nc.vector.tensor_mul(
                                    PTs[h2][:, kt, off:off + P],
                                    PTs[h2][:, kt, off:off + P],
                                    cmask01[:])
                    # AV token-major, fused row-sum via the ones column
                    for qt in range(qc * 4, qc * 4 + 4):
                        qoff = qt * P - qc * 512
                        for h2 in range(HPC):
                            hs = h2 * HD
                            ps_y = psY.tile([P, HD + 1], dt.float32, space="PSUM",
                                            name="ps_y", tag="small")
                            for kt in range(qt + 1):
                                nc.tensor.matmul(
                                    ps_y[:], lhsT=PTs[h2][:, kt, qoff:qoff + P],
                                    rhs=v_tok[:, kt, h2, :],
                                    start=(kt == 0), stop=(kt == qt))
                            rec = app.tile([P, 1], dt.float32, name="rec", tag="rec",
                                           bufs=3)
                            nc.vector.reciprocal(rec[:], ps_y[:, HD:HD + 1])
                            nc.vector.tensor_scalar_mul(
                                y_sb[:, qt, hs:hs + HD], ps_y[:, 0:HD], rec[:, 0:1])
                    for qt in range(qc * 4, qc * 4 + 4):
                        ps_t = psY.tile([P, P], dt.bfloat16, space="PSUM",
                                        name="ps_t", tag="small")
                        nc.tensor.transpose(ps_t[:], y_sb[:, qt, :], ident_bf[:])
                        nc.vector.tensor_copy(yT[:, qt * P:(qt + 1) * P], ps_t[:])
                return yT

            def proj_partial(yT, wproj, half, name):
                """Partial attn output, feature-major, blocked by destination
                core: rs_in [NCORES, P, DT, TPC] bf16."""
                rs_in = dramp.tile([NCORES, P, DT, TPC], dt.bfloat16,
                                   name=f"rsin_{name}", tag="rsin")
                for tk in range(NT):
                    prd = app.tile([P, DT, 512], dt.bfloat16, name=f"prd{half}_{tk}",
                                   tag="prd")
                    for dc in range(DT):
                        ps = psA.tile([P, 512], dt.float32, space="PSUM",
                                      name="ps_pr", tag="psA")
                        nc.tensor.matmul(
                            ps[:], lhsT=wproj[:, dc * P:(dc + 1) * P],
                            rhs=yT[:, tk * 512:(tk + 1) * 512], start=True, stop=True)
                        if dc % 2 == 0:
                            nc.vector.tensor_copy(prd[:, dc, :], ps[:])
                        else:
                            nc.scalar.copy(prd[:, dc, :], ps[:])
                    for tb in range(tk * 4, (tk + 1) * 4):
                        nc.sync.dma_start(
                            rs_in[tb], prd[:, :, (tb - tk * 4) * P:(tb - tk * 4 + 1) * P])
                return rs_in

            def mlp_half(hn2, li, half):
                """Token-local MLP for one half (128 tokens) with streamed
                bf16 fc weights. Run per half so one half's MLP hides the
                other half's collectives."""
                mTm = app.tile([P, NFC, TPC], dt.bfloat16, name=f"mTm{li}_{half}",
                               tag="mTm")
                b1 = bfc1t(li)
                for fc in range(NFC):
                    wf1c = wp.tile([P, DT, P], dt.bfloat16,
                                   name=f"wf1c{li}_{half}_{fc}", tag="wf1c",
                                   bufs=4)
                    nc.sync.dma_start(wf1c[:], wfc1_p[li, fc])
                    ps = psA.tile([P, TPC], dt.float32, space="PSUM",
                                  name="ps_f1", tag="psA")
                    for dti in range(DT):
                        nc.tensor.matmul(
                            ps[:], lhsT=wf1c[:, dti, :], rhs=hn2[:, dti, :],
                            start=(dti == 0), stop=(dti == DT - 1))
                    nc.scalar.activation(
                        mTm[:, fc, :], ps[:], AF.Gelu, bias=b1[:, fc:fc + 1])
                b2 = bfc2t(li)
                for dc in range(DT):
                    wf2c = wp.tile([P, NFC, P], dt.bfloat16,
                                   name=f"wf2c{li}_{half}_{dc}", tag="wf2c",
                                   bufs=2)
                    nc.sync.dma_start(wf2c[:], wfc2_p[li, dc])
                    ps2 = psA.tile([P, TPC], dt.float32, space="PSUM",
                                   name="ps_f2", tag="psA")
                    for kt in range(NFC):
                        nc.tensor.matmul(
                            ps2[:], lhsT=wf2c[:, kt, :], rhs=mTm[:, kt, :],
                            start=(kt == 0), stop=(kt == NFC - 1))
                    nc.vector.scalar_tensor_tensor(
                        out=hres[half][:, dc, :], in0=ps2[:],
                        scalar=b2[:, dc:dc + 1],
                        in1=hres[half][:, dc, :], op0=OP.add, op1=OP.add)

            # ---------------- transformer layers ----------------
            # The two batch halves are software-pipelined with an explicit
            # 4-stage emission order (engine queues are strict FIFO, so the
            # static order decides what can cover a collective's latency):
            #   A: h0 qkv/attn/proj, RS(h0) trigger   <- covered by B
            #   B: h1 qkv/attn/proj, RS(h1) trigger   <- covers RS(h0)
            #   C: h0 residual/ln2/MLP/ln1, AG(h0)    <- covers RS(h1)
            #   D: h1 residual/ln2/MLP/ln1, AG(h1)    <- covers AG(h0); AG(h1)
            #      is covered by the next layer's stage A.
            ag = []
            for h in range(B):
                hn0 = ln_normalize(hres[h], f"pro{h}")
                ag.append(allgather_read(hn0, f"l0a{h}"))
            for li in range(L):
                wqkv, wproj = load_weights(li)
                rs_outs = [None, None]
                for h in range(B):
                    qkT = qkv_block(ag[h], wqkv, li, h)
                    yT = attention(qkT, h)
                    rs_in = proj_partial(yT, wproj, h, f"l{li}p{h}")
                    rs_outs[h] = rs_start(rs_in, f"l{li}p{h}")
                ag_next = [None, None]
                for h in range(B):
                    rs_finish(rs_outs[h], li, hres[h], f"l{li}p{h}")
                    hn2 = ln_normalize(hres[h], f"l{li}m{h}")
                    mlp_half(hn2, li, h)
                    # trailing LN + AllGather feeds the next layer (or, after
                    # the last layer, the folded final LN for the LM head)
                    hn1 = ln_normalize(hres[h], f"l{li}n{h}")
                    ag_next[h] = allgather_read(hn1, f"l{li + 1}a{h}")
                ag = ag_next

            # ---------------- LM head ----------------
            afTs = ag
            for mc in range(NMC):
                wlm = wp.tile([P, DT, MC], dt.bfloat16, name=f"wlm{mc}", tag="wlm",
                              bufs=3)
                nc.sync.dma_start(wlm[:], wlm_p[mc])
                # per pass: two 512-token chunks accumulate in two PSUM banks;
                # consecutive matmuls share the same stationary wlm slice so
                # the LDWEIGHTS pipeline stays fed
                for hp in range(B):
                    psL = [psA.tile([MC, 512], dt.float32, space="PSUM",
                                    name=f"ps_lm{mc}_{hp}_{i}", tag="psA")
                           for i in range(2)]
                    for dti in range(DT):
                        for tq in range(2):
                            nc.tensor.matmul(
                                psL[tq][:],
                                lhsT=wlm[:, dti, :],
                                rhs=afTs[hp][tq][:, :, dti, :],
                                start=(dti == 0), stop=(dti == DT - 1))
                    for tq in range(2):
                        lsb = app.tile([MC, 512], dt.float32, name="lsb",
                                       tag="lsb", bufs=4)
                        if tq == 0:
                            nc.vector.tensor_scalar_add(
                                lsb[:], psL[tq][:], blm_sb[:, mc:mc + 1])
                        else:
                            nc.scalar.add(lsb[:], psL[tq][:], blm_sb[:, mc:mc + 1])
                        nc.gpsimd.dma_start(
                            logits_p[mc * MC:(mc + 1) * MC,
                                     hp * T + tq * 512:hp * T + (tq + 1) * 512],
                            lsb[:])

    nc.compile()
    return nc


def _get_nc():
    no_coll = os.environ.get("KERNEL_NO_COLL", "0") == "1"
    key = ("nc", no_coll)
    if key not in _CACHE:
        _CACHE[key] = _build_nc(no_coll)
    return _CACHE[key]


def build_in_maps(input_ids, wte, wpe, ln1_g, ln1_b, w_qkv, b_qkv, w_proj,
                  b_proj, ln2_g, ln2_b, w_fc1, b_fc1, w_fc2, b_fc2, lnf_g,
                  lnf_b, w_lm):
    import ml_dtypes
    f32 = np.float32
    bf16 = ml_dtypes.bfloat16

    ids = np.asarray(input_ids).astype(np.int64)
    wte = np.asarray(wte, dtype=f32)
    wpe = np.asarray(wpe, dtype=f32)
    g1 = np.asarray(ln1_g, f32)
    b1 = np.asarray(ln1_b, f32)
    g2 = np.asarray(ln2_g, f32)
    b2 = np.asarray(ln2_b, f32)
    gf = np.asarray(lnf_g, f32)
    bf = np.asarray(lnf_b, f32)
    Wq = np.asarray(w_qkv, f32)
    Wp = np.asarray(w_proj, f32)
    W1 = np.asarray(w_fc1, f32)
    W2 = np.asarray(w_fc2, f32)
    Wlm = np.asarray(w_lm, f32)
    bq = np.asarray(b_qkv, f32)
    bp = np.asarray(b_proj, f32)
    bb1 = np.asarray(b_fc1, f32)
    bb2 = np.asarray(b_fc2, f32)

    # fold LN gains into consuming weights; betas into their biases
    Wq_f = Wq * g1[:, :, None]                       # [L, D, 3D]
    bq_f = np.einsum('ld,ldo->lo', b1, Wq) + bq      # [L, 3D]
    W1_f = W1 * g2[:, :, None]                       # [L, D, 4D]
    b1_f = np.einsum('ld,ldo->lo', b2, W1) + bb1     # [L, 4D]
    Wlm_f = Wlm * gf[:, None]                        # [D, V]
    blm_f = bf @ Wlm                                 # [V]

    # embeddings, feature-major per core
    emb = wte[ids] + wpe[None, :, :]                 # [B, T, D]

    in_maps = []
    for r in range(NCORES):
        t0, t1 = r * TPC, (r + 1) * TPC
        cols = np.r_[P * r:P * r + P, D + P * r:D + P * r + P,
                     2 * D + P * r:2 * D + P * r + P]
        vs, ve = r * VPC, (r + 1) * VPC

        # emb_fm [B, P, DT, TPC]
        e = emb[:, t0:t1, :]                         # [B, TPC, D]
        emb_fm = np.ascontiguousarray(
            e.transpose(0, 2, 1).reshape(B, DT, P, TPC).transpose(0, 2, 1, 3))

        # wqkv [L, P, DT, QKVC]
        wq = Wq_f[:, :, cols]                        # [L, D, 384]
        wq = wq.reshape(L, DT, P, QKVC).transpose(0, 2, 1, 3)

        # wproj [L, P, D] (rows P*r..P*r+P)
        wpj = Wp[:, P * r:P * r + P, :]

        # wfc1 [L, NFC, P, DT, P]
        w1 = W1_f.reshape(L, DT, P, NFC, P).transpose(0, 3, 2, 1, 4)

        # wfc2 [L, DT, P, NFC, P]
        w2 = W2.reshape(L, NFC, P, DT, P).transpose(0, 3, 2, 1, 4)

        # wlm [NMC, P, DT, MC]
        wl = Wlm_f[:, vs:ve].reshape(DT, P, NMC, MC).transpose(2, 1, 0, 3)

        # bias_all [L, P, 3 + NFC + DT + DT]
        bias_all = np.concatenate([
            bq_f[:, cols].reshape(L, 3, P).transpose(0, 2, 1),
            b1_f.reshape(L, NFC, P).transpose(0, 2, 1),
            bp.reshape(L, DT, P).transpose(0, 2, 1),
            bb2.reshape(L, DT, P).transpose(0, 2, 1),
        ], axis=2)

        m = {
            "emb_fm": emb_fm,
            "wqkv": np.ascontiguousarray(wq.astype(bf16)),
            "wproj": np.ascontiguousarray(wpj.astype(bf16)),
            "wfc1": np.ascontiguousarray(w1.astype(bf16)),
            "wfc2": np.ascontiguousarray(w2.astype(bf16)),
            "wlm": np.ascontiguousarray(wl.astype(bf16)),
            "bias_all": np.ascontiguousarray(bias_all),
            "blm": np.ascontiguousarray(
                blm_f[vs:ve].reshape(NMC, MC).T.astype(f32)),
        }
        in_maps.append(m)

    return in_maps


def kernel(**inputs):
    global last_exec_time_ns
    from concourse.bass_utils import run_bass_kernel_spmd

    in_maps = build_in_maps(**inputs)
    nc = _get_nc()
    trace = os.environ.get("KERNEL_TRACE", "0") == "1"
    res = run_bass_kernel_spmd(nc, in_maps, list(range(NCORES)), trace=trace)
    last_exec_time_ns = res.exec_time_ns

    parts = [res.results[r]["logits"] for r in range(NCORES)]  # [VPC, B*T] each
    full = np.concatenate(parts, axis=0)          # [V, B*T]
    out = full.T.reshape(B, T, V).astype(np.float32)
    return out
